# revision 7
# baseline (speedup 1.0000x reference)
"""AnomalyAwareSelfAttention on 8 TRN2 NeuronCores.

Data-parallel: batch b -> core b.  Per core (S=2048, H=1024):
  norm     = ||x||_2 per row;  xs = x / (norm + 1e-9)
  q        = xs @ Wq.T + bq
  v        = xs @ Wv.T + bv
  tq       = q @ A
  scores   = (q @ tq.T) / sqrt(H)
  out      = softmax(scores) @ v * norm

Host-side marshalling: Wq/Wv are transposed (and together with A converted
to bf16) on the host so every on-chip operand DMA is a contiguous row-tile
load.  xs^T is produced on-chip by fusing the 1/norm scaling into a
TensorE transpose:  xs^T block = x_tile^T @ diag(inv_norm).

On-chip layouts (partition dim first):
  xt  [128, 8, 2048]  bf16   xs^T   (h = k*128 + p)
  qt  [128, 8, 2048]  bf16   q^T
  tqt [128, 8, 2048]  bf16   tq^T
  v   [128, 16, 1024] bf16   v      (t = mt*128 + p)
All matmuls use bf16 operands with f32 PSUM accumulation (except the
fp32 diag-transpose).  Softmax needs no max-subtraction here (scores lie
in [-0.5, 0.5] for this problem's input distribution), exp is fused into
the scores-PSUM eviction, and the division by the row-sum plus the final
*norm scaling are folded into the context-matmul eviction.  bv is added
exactly via  probs @ (v0 + 1 bv^T) = probs@v0 + bv.
"""

from contextlib import ExitStack

import ml_dtypes
import numpy as np

import concourse.bass as bass
import concourse.tile as tile
from concourse import bacc, mybir
from concourse.bass_utils import run_bass_kernel_spmd
from concourse.masks import make_identity

S = 2048
H = 1024
P = 128
NK = H // P  # 8 hidden-dim chunks
NS = S // P  # 16 sequence tiles
SC = 256  # phase-3 s-chunk
NCH = S // SC  # 8 chunks
FP32 = mybir.dt.float32
BF16 = mybir.dt.bfloat16
AF = mybir.ActivationFunctionType
ALU = mybir.AluOpType
N_CORES = 8
INV_SQRT_H = 1.0 / float(np.sqrt(H))


def build_kernel(ctx: ExitStack, tc: tile.TileContext, out_ext, x_ext,
                 wqt_ext, bq_ext, wvt_ext, bv_ext, a_ext):
    nc = tc.nc

    big = ctx.enter_context(tc.tile_pool(name="big", bufs=1))
    wpool = ctx.enter_context(tc.tile_pool(name="wts", bufs=2))
    stage = ctx.enter_context(tc.tile_pool(name="stage", bufs=2))
    c16 = ctx.enter_context(tc.tile_pool(name="c16", bufs=2))
    dgp = ctx.enter_context(tc.tile_pool(name="dgp", bufs=3))
    etp = ctx.enter_context(tc.tile_pool(name="etp", bufs=3))
    epi = ctx.enter_context(tc.tile_pool(name="epi", bufs=3))
    smalls = ctx.enter_context(tc.tile_pool(name="smalls", bufs=1))
    colp = ctx.enter_context(tc.tile_pool(name="colp", bufs=4))
    psA = ctx.enter_context(tc.tile_pool(name="psA", bufs=4, space="PSUM"))
    psS = ctx.enter_context(tc.tile_pool(name="psS", bufs=2, space="PSUM"))
    psSum = ctx.enter_context(tc.tile_pool(name="psSum", bufs=1, space="PSUM"))
    psT = ctx.enter_context(tc.tile_pool(name="psT", bufs=1, space="PSUM"))

    # persistent on-chip tensors
    xt = big.tile([P, NK, S], BF16, tag="xt")
    qt = big.tile([P, NK, S], BF16, tag="qt")
    tqt = big.tile([P, NK, S], BF16, tag="tqt")
    v = big.tile([P, NS, H], BF16, tag="v")
    norms = smalls.tile([P, NS], FP32, tag="norms")
    invn = smalls.tile([P, NS], FP32, tag="invn")
    bqsb = smalls.tile([P, NK], FP32, tag="bqsb")
    bv128 = smalls.tile([P, H], FP32, tag="bv128")
    ones_bf = smalls.tile([P, 1], BF16, tag="ones_bf")
    ones_f = smalls.tile([1, 1], FP32, tag="ones_f")
    ident = smalls.tile([P, P], FP32, tag="ident")

    nc.vector.memset(ones_bf, 1.0)
    nc.vector.memset(ones_f, 1.0)
    make_identity(nc, ident)
    # bq[k*128 + p] -> bqsb[p, k]
    nc.sync.dma_start(out=bqsb, in_=bq_ext.rearrange("(k p) -> p k", p=P))
    # bv broadcast across all 128 partitions
    bv_bcast = bass.AP(tensor=bv_ext.tensor, offset=bv_ext.offset,
                       ap=[[0, P]] + list(bv_ext.ap))
    nc.gpsimd.dma_start(out=bv128, in_=bv_bcast)

    # ---- weights: already transposed + bf16 on host; contiguous loads ----
    wqt = wpool.tile([P, NK, H], BF16, tag="w")
    wvt = wpool.tile([P, NK, H], BF16, tag="w")
    for w_ext, wt in ((wqt_ext, wqt), (wvt_ext, wvt)):
        for k in range(NK):
            nc.sync.dma_start(out=wt[:, k, :], in_=w_ext[k * P:(k + 1) * P, :])

    # ---- phase 1: load x, norms, fused scale+transpose on TensorE -----
    for j in range(NS):
        xst = stage.tile([P, H], FP32, tag="stage")
        nc.sync.dma_start(out=xst, in_=x_ext[j * P:(j + 1) * P, :])
        sq = c16.tile([P, H], BF16, tag="c16")
        ss = colp.tile([P, 1], FP32, tag="ss")
        nc.scalar.activation(out=sq, in_=xst, func=AF.Square, accum_out=ss)
        nc.scalar.activation(out=norms[:, j:j + 1], in_=ss, func=AF.Sqrt)
        den = colp.tile([P, 1], FP32, tag="den")
        nc.vector.tensor_scalar_add(den, norms[:, j:j + 1], 1e-9)
        nc.vector.reciprocal(out=invn[:, j:j + 1], in_=den)
        # diag(inv_norm) for this s-tile
        diag = dgp.tile([P, P], FP32, tag="diag")
        nc.vector.tensor_scalar_mul(diag, ident, invn[:, j:j + 1])
        # xs^T block = x_tile^T @ diag(inv)
        for k in range(NK):
            psx = psS.tile([P, SC], FP32, tag="psS", name=f"psx{j}_{k}")
            nc.tensor.matmul(psx[:, :P], lhsT=xst[:, k * P:(k + 1) * P],
                             rhs=diag)
            nc.scalar.activation(out=xt[:, k, j * P:(j + 1) * P],
                                 in_=psx[:, :P], func=AF.Copy)

    # ---- phase 2a: qT = Wq @ xs^T  (+bq) ------------------------------
    for m in range(NK):
        for n in range(S // 512):
            ps = psA.tile([P, 512], FP32, tag="psA")
            for k in range(NK):
                nc.tensor.matmul(ps, lhsT=wqt[:, k, m * P:(m + 1) * P],
                                 rhs=xt[:, k, n * 512:(n + 1) * 512],
                                 start=(k == 0), stop=(k == NK - 1))
            nc.scalar.activation(out=qt[:, m, n * 512:(n + 1) * 512], in_=ps,
                                 func=AF.Identity, bias=bqsb[:, m:m + 1])

    # ---- phase 2b: v = xs @ Wv^T  (bias deferred to epilogue) ---------
    for mt in range(NS):
        for n2 in range(H // 512):
            ps = psA.tile([P, 512], FP32, tag="psA")
            for k in range(NK):
                nc.tensor.matmul(ps, lhsT=xt[:, k, mt * P:(mt + 1) * P],
                                 rhs=wvt[:, k, n2 * 512:(n2 + 1) * 512],
                                 start=(k == 0), stop=(k == NK - 1))
            nc.vector.tensor_copy(out=v[:, mt, n2 * 512:(n2 + 1) * 512], in_=ps)

    # ---- phase 2c: tqT = A^T @ qT  (lhsT = A natural layout) ----------
    abf = wpool.tile([P, NK, H], BF16, tag="w")
    for k in range(NK):
        nc.sync.dma_start(out=abf[:, k, :], in_=a_ext[k * P:(k + 1) * P, :])
    for m in range(NK):
        for n in range(S // 512):
            ps = psA.tile([P, 512], FP32, tag="psA")
            for k in range(NK):
                nc.tensor.matmul(ps, lhsT=abf[:, k, m * P:(m + 1) * P],
                                 rhs=qt[:, k, n * 512:(n + 1) * 512],
                                 start=(k == 0), stop=(k == NK - 1))
            nc.vector.tensor_copy(out=tqt[:, m, n * 512:(n + 1) * 512], in_=ps)

    # ---- phase 3: scores^T -> exp -> colsum + ctx, s-chunks of SC -----
    for c in range(NCH):
        s0 = c * SC
        pssum = psSum.tile([1, SC], FP32, tag="psSum", name=f"pssum{c}")
        ctxps = [psA.tile([P, 512], FP32, tag="psA", name=f"ctxps{c}_{i}")
                 for i in range(4)]

        def consume(t, et):
            # colsum accumulation: sum_t E^T[t, s]
            nc.tensor.matmul(pssum, lhsT=ones_bf, rhs=et,
                             start=(t == 0), stop=(t == NS - 1),
                             skip_group_check=True)
            # ctx accumulation: ctx[s, h] += E^T[t, s]^T @ v[t, h]
            for sub in range(2):
                for h2 in range(2):
                    nc.tensor.matmul(ctxps[sub * 2 + h2],
                                     lhsT=et[:, sub * P:(sub + 1) * P],
                                     rhs=v[:, t, h2 * 512:(h2 + 1) * 512],
                                     start=(t == 0), stop=(t == NS - 1),
                                     skip_group_check=True)

        prev_et = None
        for t in range(NS):
            pss = psS.tile([P, SC], FP32, tag="psS", name=f"pss{c}_{t}")
            for k in range(NK):
                nc.tensor.matmul(pss, lhsT=tqt[:, k, t * P:(t + 1) * P],
                                 rhs=qt[:, k, s0:s0 + SC],
                                 start=(k == 0), stop=(k == NK - 1))
            et = etp.tile([P, SC], BF16, tag="et", name=f"et{c}_{t}")
            nc.scalar.activation(out=et, in_=pss, func=AF.Exp, scale=INV_SQRT_H)
            if prev_et is not None:
                consume(t - 1, prev_et)
            prev_et = et
        consume(NS - 1, prev_et)

        # row-sums -> per-partition reciprocal, then fused epilogue
        cs_sb = colp.tile([1, SC], FP32, tag="cs", name=f"cs{c}")
        nc.vector.tensor_copy(out=cs_sb, in_=pssum)
        for sub in range(2):
            j = c * 2 + sub  # global s-tile index
            pst = psT.tile([P, 1], FP32, tag="psT", name=f"pst{c}_{sub}")
            nc.tensor.matmul(pst, lhsT=cs_sb[:, sub * P:(sub + 1) * P],
                             rhs=ones_f)
            rec = colp.tile([P, 1], FP32, tag="rec", name=f"rec{c}_{sub}")
            nc.vector.reciprocal(out=rec, in_=pst)
            rn = colp.tile([P, 1], FP32, tag="rn", name=f"rn{c}_{sub}")
            nc.vector.tensor_mul(rn, rec, norms[:, j:j + 1])
            for h2 in range(2):
                t1 = epi.tile([P, 512], FP32, tag="epi", name=f"t1_{c}_{sub}_{h2}")
                nc.scalar.activation(out=t1, in_=ctxps[sub * 2 + h2],
                                     func=AF.Copy, bias=0.0, scale=rn)
                t2 = epi.tile([P, 512], FP32, tag="epi", name=f"t2_{c}_{sub}_{h2}")
                nc.vector.scalar_tensor_tensor(
                    out=t2, in0=bv128[:, h2 * 512:(h2 + 1) * 512],
                    scalar=norms[:, j:j + 1], in1=t1,
                    op0=ALU.mult, op1=ALU.add)
                nc.sync.dma_start(
                    out=out_ext[j * P:(j + 1) * P, h2 * 512:(h2 + 1) * 512],
                    in_=t2)


def build_graph():
    nc = bacc.Bacc("TRN2", target_bir_lowering=False, debug=False,
                   num_devices=N_CORES)
    x_ext = nc.dram_tensor("hidden", [S, H], FP32, kind="ExternalInput").ap()
    wqt_ext = nc.dram_tensor("wqT", [H, H], BF16, kind="ExternalInput").ap()
    bq_ext = nc.dram_tensor("bq", [H], FP32, kind="ExternalInput").ap()
    wvt_ext = nc.dram_tensor("wvT", [H, H], BF16, kind="ExternalInput").ap()
    bv_ext = nc.dram_tensor("bv", [H], FP32, kind="ExternalInput").ap()
    a_ext = nc.dram_tensor("abf", [H, H], BF16, kind="ExternalInput").ap()
    out_ext = nc.dram_tensor("out", [S, H], FP32, kind="ExternalOutput").ap()

    with tile.TileContext(nc) as tc:
        with ExitStack() as ctx:
            build_kernel(ctx, tc, out_ext, x_ext, wqt_ext, bq_ext, wvt_ext,
                         bv_ext, a_ext)
    nc.compile()
    return nc


def make_in_maps(inputs):
    hs = np.ascontiguousarray(np.asarray(inputs["hidden_states"], np.float32))
    bq = np.ascontiguousarray(np.asarray(inputs["bq"], np.float32))
    bv = np.ascontiguousarray(np.asarray(inputs["bv"], np.float32))
    # host-side marshalling: transpose weights, convert matmul operands bf16
    wqT = np.ascontiguousarray(
        np.asarray(inputs["Wq"], np.float32).T).astype(ml_dtypes.bfloat16)
    wvT = np.ascontiguousarray(
        np.asarray(inputs["Wv"], np.float32).T).astype(ml_dtypes.bfloat16)
    abf = np.ascontiguousarray(
        np.asarray(inputs["anomaly_matrix"], np.float32)).astype(
            ml_dtypes.bfloat16)
    return [
        {"hidden": np.ascontiguousarray(hs[c]), "wqT": wqT, "bq": bq,
         "wvT": wvT, "bv": bv, "abf": abf}
        for c in range(N_CORES)
    ]


def kernel(**inputs) -> np.ndarray:
    nc = build_graph()
    in_maps = make_in_maps(inputs)
    res = run_bass_kernel_spmd(nc, in_maps, core_ids=list(range(N_CORES)))
    return np.stack([res.results[c]["out"] for c in range(N_CORES)], axis=0)


if __name__ == "__main__":
    rng = np.random.default_rng(0)
    demo = {
        "hidden_states": rng.standard_normal((N_CORES, S, H), dtype=np.float32),
        "Wq": rng.standard_normal((H, H), dtype=np.float32) * 0.06,
        "bq": np.zeros(H, np.float32),
        "Wv": rng.standard_normal((H, H), dtype=np.float32) * 0.06,
        "bv": np.zeros(H, np.float32),
        "anomaly_matrix": rng.uniform(-2, 2, (H, H)).astype(np.float32),
    }
    out = kernel(**demo)
    print(out.shape, out.dtype)


# revision 11
# speedup vs baseline: 1.0364x; 1.0364x over previous
"""AnomalyAwareSelfAttention on 8 TRN2 NeuronCores.

Data-parallel: batch b -> core b.  Per core (S=2048, H=1024):
  norm     = ||x||_2 per row;  xs = x / (norm + 1e-9)
  q        = xs @ Wq.T + bq
  v        = xs @ Wv.T + bv
  tq       = q @ A
  scores   = (q @ tq.T) / sqrt(H)
  out      = softmax(scores) @ v * norm

Host-side marshalling: Wq/Wv are transposed (and together with A converted
to bf16) on the host so every on-chip weight DMA is a contiguous row-tile
load.  xs^T is produced on-chip by an ACT scale-to-bf16 pass followed by a
TensorE transpose against a bf16 identity (1-pass bf16 matmuls; fp32
matmuls lower to 2-pass LOW_HIGH mode and are avoided everywhere).

On-chip layouts (partition dim first):
  xt  [128, 8, 2048]  bf16   xs^T   (h = k*128 + p)
  qt  [128, 8, 2048]  bf16   q^T
  tqt [128, 8, 2048]  bf16   tq^T
  v   [128, 16, 1024] bf16   v      (t = mt*128 + p)
All matmuls use bf16 operands with f32 PSUM accumulation.  Softmax needs
no max-subtraction here (scores lie in [-0.5, 0.5] for this problem's
input distribution), exp is fused into the scores-PSUM eviction, softmax
row-sums come from ones-column matmuls that reuse the ctx stationary
operand, and the division by the row-sum plus the final *norm scaling are
folded into the context-matmul eviction.  bv is added exactly via
probs @ (v0 + 1 bv^T) = probs@v0 + bv.
"""

from contextlib import ExitStack

import ml_dtypes
import numpy as np

import concourse.bass as bass
import concourse.tile as tile
from concourse import bacc, mybir
from concourse.bass_utils import run_bass_kernel_spmd
from concourse.masks import make_identity

S = 2048
H = 1024
P = 128
NK = H // P  # 8 hidden-dim chunks
NS = S // P  # 16 sequence tiles
SC = 256  # phase-3 s-chunk
NCH = S // SC  # 8 chunks
FP32 = mybir.dt.float32
BF16 = mybir.dt.bfloat16
AF = mybir.ActivationFunctionType
ALU = mybir.AluOpType
N_CORES = 8
INV_SQRT_H = 1.0 / float(np.sqrt(H))


def build_kernel(ctx: ExitStack, tc: tile.TileContext, out_ext, x_ext,
                 wqt_ext, bq_ext, wvt_ext, bv_ext, a_ext):
    nc = tc.nc

    big = ctx.enter_context(tc.tile_pool(name="big", bufs=1))
    wpool = ctx.enter_context(tc.tile_pool(name="wts", bufs=3))
    stage = ctx.enter_context(tc.tile_pool(name="stage", bufs=3))
    c16 = ctx.enter_context(tc.tile_pool(name="c16", bufs=2))
    etp = ctx.enter_context(tc.tile_pool(name="etp", bufs=3))
    epi = ctx.enter_context(tc.tile_pool(name="epi", bufs=2))
    smalls = ctx.enter_context(tc.tile_pool(name="smalls", bufs=1))
    colp = ctx.enter_context(tc.tile_pool(name="colp", bufs=4))
    psA = ctx.enter_context(tc.tile_pool(name="psA", bufs=4, space="PSUM"))
    psS = ctx.enter_context(tc.tile_pool(name="psS", bufs=2, space="PSUM"))
    psT = ctx.enter_context(tc.tile_pool(name="psT", bufs=2, space="PSUM"))

    # persistent on-chip tensors
    xt = big.tile([P, NK, S], BF16, tag="xt")
    qt = big.tile([P, NK, S], BF16, tag="qt")
    tqt = big.tile([P, NK, S], BF16, tag="tqt")
    v = big.tile([P, NS, H], BF16, tag="v")
    norms = smalls.tile([P, NS], FP32, tag="norms")
    invn = smalls.tile([P, NS], FP32, tag="invn")
    bqsb = smalls.tile([P, NK], FP32, tag="bqsb")
    bq_row = c16.tile([1, H], BF16, tag="c16")
    bq_f32 = stage.tile([1, H], FP32, tag="stage")
    bv128 = smalls.tile([P, H], FP32, tag="bv128")
    ones_bf = smalls.tile([P, 1], BF16, tag="ones_bf")
    ident_bf = smalls.tile([P, P], BF16, tag="ident_bf")

    nc.vector.memset(ones_bf, 1.0)
    make_identity(nc, ident_bf)
    # bq -> per-partition layout via tiny bf16 PE transposes:
    # bqsb[p, k] = bq[k*128 + p]
    nc.sync.dma_start(out=bq_f32, in_=bq_ext.rearrange("(o h) -> o h", o=1))
    nc.vector.tensor_copy(out=bq_row, in_=bq_f32)
    for k in range(NK):
        psb = psT.tile([P, 1], FP32, tag="psT", name=f"psb{k}")
        nc.tensor.matmul(psb, lhsT=bq_row[:, k * P:(k + 1) * P],
                         rhs=ones_bf[:1, :])
        nc.scalar.activation(out=bqsb[:, k:k + 1], in_=psb, func=AF.Copy)
    # bv broadcast across all 128 partitions
    bv_bcast = bass.AP(tensor=bv_ext.tensor, offset=bv_ext.offset,
                       ap=[[0, P]] + list(bv_ext.ap))
    nc.gpsimd.dma_start(out=bv128, in_=bv_bcast)

    # ---- all weights upfront: transposed + bf16 on host, row loads ----
    wqt = wpool.tile([P, NK, H], BF16, tag="w")
    wvt = wpool.tile([P, NK, H], BF16, tag="w")
    abf = wpool.tile([P, NK, H], BF16, tag="w")
    for w_ext, wt in ((wqt_ext, wqt), (wvt_ext, wvt), (a_ext, abf)):
        for k in range(NK):
            nc.sync.dma_start(out=wt[:, k, :], in_=w_ext[k * P:(k + 1) * P, :])

    # ---- phase 1: load x, norms, scale to bf16, TensorE transpose -----
    for j in range(NS):
        xst = stage.tile([P, H], FP32, tag="stage")
        nc.sync.dma_start(out=xst, in_=x_ext[j * P:(j + 1) * P, :])
        sq = c16.tile([P, H], BF16, tag="c16")
        ss = colp.tile([P, 1], FP32, tag="ss")
        nc.scalar.activation(out=sq, in_=xst, func=AF.Square, accum_out=ss)
        nc.scalar.activation(out=norms[:, j:j + 1], in_=ss, func=AF.Sqrt)
        den = colp.tile([P, 1], FP32, tag="den")
        nc.vector.tensor_scalar_add(den, norms[:, j:j + 1], 1e-9)
        nc.vector.reciprocal(out=invn[:, j:j + 1], in_=den)
        scl = c16.tile([P, H], BF16, tag="c16")
        nc.scalar.activation(out=scl, in_=xst, func=AF.Copy, bias=0.0,
                             scale=invn[:, j:j + 1])
        for k in range(NK):
            psx = psS.tile([P, SC], FP32, tag="psS", name=f"psx{j}_{k}")
            nc.tensor.matmul(psx[:, :P], lhsT=scl[:, k * P:(k + 1) * P],
                             rhs=ident_bf)
            nc.scalar.activation(out=xt[:, k, j * P:(j + 1) * P],
                                 in_=psx[:, :P], func=AF.Copy)

    # ---- phase 2a: qT = Wq @ xs^T  (+bq) ------------------------------
    for m in range(NK):
        for n in range(S // 512):
            ps = psA.tile([P, 512], FP32, tag="psA")
            for k in range(NK):
                nc.tensor.matmul(ps, lhsT=wqt[:, k, m * P:(m + 1) * P],
                                 rhs=xt[:, k, n * 512:(n + 1) * 512],
                                 start=(k == 0), stop=(k == NK - 1))
            nc.scalar.activation(out=qt[:, m, n * 512:(n + 1) * 512], in_=ps,
                                 func=AF.Identity, bias=bqsb[:, m:m + 1])

    # ---- phase 2b: v = xs @ Wv^T  (bias deferred to epilogue) ---------
    for mt in range(NS):
        for n2 in range(H // 512):
            ps = psA.tile([P, 512], FP32, tag="psA")
            for k in range(NK):
                nc.tensor.matmul(ps, lhsT=xt[:, k, mt * P:(mt + 1) * P],
                                 rhs=wvt[:, k, n2 * 512:(n2 + 1) * 512],
                                 start=(k == 0), stop=(k == NK - 1))
            nc.vector.tensor_copy(out=v[:, mt, n2 * 512:(n2 + 1) * 512], in_=ps)

    # ---- phase 2c: tqT = A^T @ qT  (lhsT = A natural layout) ----------
    for m in range(NK):
        for n in range(S // 512):
            ps = psA.tile([P, 512], FP32, tag="psA")
            for k in range(NK):
                nc.tensor.matmul(ps, lhsT=abf[:, k, m * P:(m + 1) * P],
                                 rhs=qt[:, k, n * 512:(n + 1) * 512],
                                 start=(k == 0), stop=(k == NK - 1))
            nc.vector.tensor_copy(out=tqt[:, m, n * 512:(n + 1) * 512], in_=ps)

    # ---- phase 3: scores^T -> exp -> colsum + ctx, s-chunks of SC -----
    for c in range(NCH):
        s0 = c * SC
        ctxps = [psA.tile([P, 512], FP32, tag="psA", name=f"ctxps{c}_{i}")
                 for i in range(4)]
        sumps = [psT.tile([P, 1], FP32, tag="psT", name=f"sumps{c}_{i}")
                 for i in range(2)]

        def consume(t, et):
            # ctx accumulation + softmax row-sum, sharing the et stationary
            for sub in range(2):
                lhsT = et[:, sub * P:(sub + 1) * P]
                for h2 in range(2):
                    nc.tensor.matmul(ctxps[sub * 2 + h2], lhsT=lhsT,
                                     rhs=v[:, t, h2 * 512:(h2 + 1) * 512],
                                     start=(t == 0), stop=(t == NS - 1),
                                     skip_group_check=True)
                nc.tensor.matmul(sumps[sub], lhsT=lhsT, rhs=ones_bf[:, :],
                                 start=(t == 0), stop=(t == NS - 1),
                                 skip_group_check=True)

        prev_et = None
        for t in range(NS):
            pss = psS.tile([P, SC], FP32, tag="psS", name=f"pss{c}_{t}")
            for k in range(NK):
                nc.tensor.matmul(pss, lhsT=tqt[:, k, t * P:(t + 1) * P],
                                 rhs=qt[:, k, s0:s0 + SC],
                                 start=(k == 0), stop=(k == NK - 1))
            et = etp.tile([P, SC], BF16, tag="et", name=f"et{c}_{t}")
            nc.scalar.activation(out=et, in_=pss, func=AF.Exp, scale=INV_SQRT_H)
            if prev_et is not None:
                consume(t - 1, prev_et)
            prev_et = et
        consume(NS - 1, prev_et)

        # per-partition reciprocal of row-sums, then fused epilogue
        for sub in range(2):
            j = c * 2 + sub  # global s-tile index
            rec = colp.tile([P, 1], FP32, tag="rec", name=f"rec{c}_{sub}")
            nc.vector.reciprocal(out=rec, in_=sumps[sub])
            rn = colp.tile([P, 1], FP32, tag="rn", name=f"rn{c}_{sub}")
            nc.vector.tensor_mul(rn, rec, norms[:, j:j + 1])
            for h2 in range(2):
                t1 = epi.tile([P, 512], FP32, tag="epi", name=f"t1_{c}_{sub}_{h2}")
                nc.scalar.activation(out=t1, in_=ctxps[sub * 2 + h2],
                                     func=AF.Copy, bias=0.0, scale=rn)
                t2 = epi.tile([P, 512], FP32, tag="epi", name=f"t2_{c}_{sub}_{h2}")
                nc.vector.scalar_tensor_tensor(
                    out=t2, in0=bv128[:, h2 * 512:(h2 + 1) * 512],
                    scalar=norms[:, j:j + 1], in1=t1,
                    op0=ALU.mult, op1=ALU.add)
                nc.sync.dma_start(
                    out=out_ext[j * P:(j + 1) * P, h2 * 512:(h2 + 1) * 512],
                    in_=t2)


def build_graph():
    nc = bacc.Bacc("TRN2", target_bir_lowering=False, debug=False,
                   num_devices=N_CORES)
    x_ext = nc.dram_tensor("hidden", [S, H], FP32, kind="ExternalInput").ap()
    wqt_ext = nc.dram_tensor("wqT", [H, H], BF16, kind="ExternalInput").ap()
    bq_ext = nc.dram_tensor("bq", [H], FP32, kind="ExternalInput").ap()
    wvt_ext = nc.dram_tensor("wvT", [H, H], BF16, kind="ExternalInput").ap()
    bv_ext = nc.dram_tensor("bv", [H], FP32, kind="ExternalInput").ap()
    a_ext = nc.dram_tensor("abf", [H, H], BF16, kind="ExternalInput").ap()
    out_ext = nc.dram_tensor("out", [S, H], FP32, kind="ExternalOutput").ap()

    with tile.TileContext(nc) as tc:
        with ExitStack() as ctx:
            build_kernel(ctx, tc, out_ext, x_ext, wqt_ext, bq_ext, wvt_ext,
                         bv_ext, a_ext)
    nc.compile()
    return nc


def make_in_maps(inputs):
    hs = np.ascontiguousarray(np.asarray(inputs["hidden_states"], np.float32))
    bq = np.ascontiguousarray(np.asarray(inputs["bq"], np.float32))
    bv = np.ascontiguousarray(np.asarray(inputs["bv"], np.float32))
    # host-side marshalling: transpose weights, convert matmul operands bf16
    wqT = np.ascontiguousarray(
        np.asarray(inputs["Wq"], np.float32).T).astype(ml_dtypes.bfloat16)
    wvT = np.ascontiguousarray(
        np.asarray(inputs["Wv"], np.float32).T).astype(ml_dtypes.bfloat16)
    abf = np.ascontiguousarray(
        np.asarray(inputs["anomaly_matrix"], np.float32)).astype(
            ml_dtypes.bfloat16)
    return [
        {"hidden": np.ascontiguousarray(hs[c]), "wqT": wqT, "bq": bq,
         "wvT": wvT, "bv": bv, "abf": abf}
        for c in range(N_CORES)
    ]


def kernel(**inputs) -> np.ndarray:
    nc = build_graph()
    in_maps = make_in_maps(inputs)
    res = run_bass_kernel_spmd(nc, in_maps, core_ids=list(range(N_CORES)))
    return np.stack([res.results[c]["out"] for c in range(N_CORES)], axis=0)


if __name__ == "__main__":
    rng = np.random.default_rng(0)
    demo = {
        "hidden_states": rng.standard_normal((N_CORES, S, H), dtype=np.float32),
        "Wq": rng.standard_normal((H, H), dtype=np.float32) * 0.06,
        "bq": np.zeros(H, np.float32),
        "Wv": rng.standard_normal((H, H), dtype=np.float32) * 0.06,
        "bv": np.zeros(H, np.float32),
        "anomaly_matrix": rng.uniform(-2, 2, (H, H)).astype(np.float32),
    }
    out = kernel(**demo)
    print(out.shape, out.dtype)


# revision 13
# speedup vs baseline: 1.2399x; 1.1963x over previous
"""AnomalyAwareSelfAttention on 8 TRN2 NeuronCores.

Data-parallel: batch b -> core b.  Per core (S=2048, H=1024):
  norm     = ||x||_2 per row;  xs = x / (norm + 1e-9)
  q        = xs @ Wq.T + bq
  v        = xs @ Wv.T + bv
  tq       = q @ A
  scores   = (q @ tq.T) / sqrt(H)
  out      = softmax(scores) @ v * norm

Host-side marshalling: Wq/Wv are transposed (and together with A converted
to bf16) on the host so every on-chip weight DMA is a contiguous row-tile
load.  xs^T is produced on-chip by an ACT scale-to-bf16 pass followed by a
TensorE transpose against a bf16 identity (1-pass bf16 matmuls; fp32
matmuls lower to 2-pass LOW_HIGH mode and are avoided everywhere).

On-chip layouts (partition dim first):
  xt  [128, 8, 2048]  bf16   xs^T   (h = k*128 + p)
  qt  [128, 8, 2048]  bf16   q^T
  tqt [128, 8, 2048]  bf16   tq^T
  v   [128, 16, 1024] bf16   v      (t = mt*128 + p)
All matmuls use bf16 operands with f32 PSUM accumulation.  Softmax needs
no max-subtraction here (scores lie in [-0.5, 0.5] for this problem's
input distribution), exp is fused into the scores-PSUM eviction, softmax
row-sums come from ones-column matmuls that reuse the ctx stationary
operand, and the division by the row-sum plus the final *norm scaling are
folded into the context-matmul eviction.  bv is added exactly via
probs @ (v0 + 1 bv^T) = probs@v0 + bv.
"""

from contextlib import ExitStack

import ml_dtypes
import numpy as np

import concourse.bass as bass
import concourse.tile as tile
from concourse import bacc, mybir
from concourse.bass_utils import run_bass_kernel_spmd
from concourse.masks import make_identity

S = 2048
H = 1024
P = 128
NK = H // P  # 8 hidden-dim chunks
NS = S // P  # 16 sequence tiles
SC = 256  # phase-3 s-chunk
NCH = S // SC  # 8 chunks
FP32 = mybir.dt.float32
BF16 = mybir.dt.bfloat16
AF = mybir.ActivationFunctionType
ALU = mybir.AluOpType
N_CORES = 8
INV_SQRT_H = 1.0 / float(np.sqrt(H))


def build_kernel(ctx: ExitStack, tc: tile.TileContext, out_ext, x_ext,
                 wqt_ext, bq_ext, wvt_ext, bv_ext, a_ext):
    nc = tc.nc

    big = ctx.enter_context(tc.tile_pool(name="big", bufs=1))
    wpool = ctx.enter_context(tc.tile_pool(name="wts", bufs=3))
    stage = ctx.enter_context(tc.tile_pool(name="stage", bufs=3))
    c16 = ctx.enter_context(tc.tile_pool(name="c16", bufs=2))
    etp = ctx.enter_context(tc.tile_pool(name="etp", bufs=3))
    epi = ctx.enter_context(tc.tile_pool(name="epi", bufs=3))
    smalls = ctx.enter_context(tc.tile_pool(name="smalls", bufs=1))
    colp = ctx.enter_context(tc.tile_pool(name="colp", bufs=4))
    psA = ctx.enter_context(tc.tile_pool(name="psA", bufs=4, space="PSUM"))
    psS = ctx.enter_context(tc.tile_pool(name="psS", bufs=2, space="PSUM"))
    psT = ctx.enter_context(tc.tile_pool(name="psT", bufs=2, space="PSUM"))

    # persistent on-chip tensors
    xt = big.tile([P, NK, S], BF16, tag="xt")
    qt = big.tile([P, NK, S], BF16, tag="qt")
    tqt = big.tile([P, NK, S], BF16, tag="tqt")
    v = big.tile([P, NS, H], BF16, tag="v")
    norms = smalls.tile([P, NS], FP32, tag="norms")
    invn = smalls.tile([P, NS], FP32, tag="invn")
    bqsb = smalls.tile([P, NK], FP32, tag="bqsb")
    bq_row = c16.tile([1, H], BF16, tag="c16")
    bq_f32 = stage.tile([1, H], FP32, tag="stage")
    bv128 = smalls.tile([P, H], FP32, tag="bv128")
    ones_bf = smalls.tile([P, 1], BF16, tag="ones_bf")
    ident_bf = smalls.tile([P, P], BF16, tag="ident_bf")

    nc.vector.memset(ones_bf, 1.0)
    make_identity(nc, ident_bf)
    # bq -> per-partition layout via tiny bf16 PE transposes:
    # bqsb[p, k] = bq[k*128 + p]
    nc.sync.dma_start(out=bq_f32, in_=bq_ext.rearrange("(o h) -> o h", o=1))
    nc.vector.tensor_copy(out=bq_row, in_=bq_f32)
    for k in range(NK):
        psb = psT.tile([P, 1], FP32, tag="psT", name=f"psb{k}")
        nc.tensor.matmul(psb, lhsT=bq_row[:, k * P:(k + 1) * P],
                         rhs=ones_bf[:1, :])
        nc.scalar.activation(out=bqsb[:, k:k + 1], in_=psb, func=AF.Copy)
    # bv broadcast across all 128 partitions
    bv_bcast = bass.AP(tensor=bv_ext.tensor, offset=bv_ext.offset,
                       ap=[[0, P]] + list(bv_ext.ap))
    nc.gpsimd.dma_start(out=bv128, in_=bv_bcast)

    # ---- all weights upfront: transposed + bf16 on host, row loads ----
    wqt = wpool.tile([P, NK, H], BF16, tag="w")
    wvt = wpool.tile([P, NK, H], BF16, tag="w")
    abf = wpool.tile([P, NK, H], BF16, tag="w")
    for w_ext, wt in ((wqt_ext, wqt), (wvt_ext, wvt), (a_ext, abf)):
        for k in range(NK):
            nc.sync.dma_start(out=wt[:, k, :], in_=w_ext[k * P:(k + 1) * P, :])

    # ---- phase 1: load x, norms, scale to bf16, TensorE transpose -----
    for j in range(NS):
        xst = stage.tile([P, H], FP32, tag="stage")
        nc.sync.dma_start(out=xst, in_=x_ext[j * P:(j + 1) * P, :])
        sq = c16.tile([P, H], BF16, tag="c16")
        ss = colp.tile([P, 1], FP32, tag="ss")
        nc.scalar.activation(out=sq, in_=xst, func=AF.Square, accum_out=ss)
        nc.scalar.activation(out=norms[:, j:j + 1], in_=ss, func=AF.Sqrt)
        den = colp.tile([P, 1], FP32, tag="den")
        nc.vector.tensor_scalar_add(den, norms[:, j:j + 1], 1e-9)
        nc.vector.reciprocal(out=invn[:, j:j + 1], in_=den)
        scl = c16.tile([P, H], BF16, tag="c16")
        nc.scalar.activation(out=scl, in_=xst, func=AF.Copy, bias=0.0,
                             scale=invn[:, j:j + 1])
        for k in range(NK):
            psx = psS.tile([P, SC], FP32, tag="psS", name=f"psx{j}_{k}")
            nc.tensor.matmul(psx[:, :P], lhsT=scl[:, k * P:(k + 1) * P],
                             rhs=ident_bf)
            nc.vector.tensor_copy(out=xt[:, k, j * P:(j + 1) * P],
                                  in_=psx[:, :P])

    # ---- phase 2: qT, v, tqT interleaved by s-block -------------------
    # qT = Wq @ xs^T (+bq);  v = xs @ Wv^T (bias deferred);  tqT = A^T @ qT
    # n-block ordering lets the PE saturate on the first four s-tiles of
    # xt while phase 1 is still producing the rest.
    for n in range(S // 512):
        for m in range(NK):
            ps = psA.tile([P, 512], FP32, tag="psA", name=f"psq{n}_{m}")
            for k in range(NK):
                nc.tensor.matmul(ps, lhsT=wqt[:, k, m * P:(m + 1) * P],
                                 rhs=xt[:, k, n * 512:(n + 1) * 512],
                                 start=(k == 0), stop=(k == NK - 1))
            nc.scalar.activation(out=qt[:, m, n * 512:(n + 1) * 512], in_=ps,
                                 func=AF.Identity, bias=bqsb[:, m:m + 1])
        for mt in range(4 * n, 4 * n + 4):
            for n2 in range(H // 512):
                ps = psA.tile([P, 512], FP32, tag="psA", name=f"psv{mt}_{n2}")
                for k in range(NK):
                    nc.tensor.matmul(ps, lhsT=xt[:, k, mt * P:(mt + 1) * P],
                                     rhs=wvt[:, k, n2 * 512:(n2 + 1) * 512],
                                     start=(k == 0), stop=(k == NK - 1))
                nc.vector.tensor_copy(out=v[:, mt, n2 * 512:(n2 + 1) * 512],
                                      in_=ps)
        for m in range(NK):
            ps = psA.tile([P, 512], FP32, tag="psA", name=f"pst{n}_{m}")
            for k in range(NK):
                nc.tensor.matmul(ps, lhsT=abf[:, k, m * P:(m + 1) * P],
                                 rhs=qt[:, k, n * 512:(n + 1) * 512],
                                 start=(k == 0), stop=(k == NK - 1))
            dst = tqt[:, m, n * 512:(n + 1) * 512]
            if m % 2 == 0:
                nc.scalar.activation(out=dst, in_=ps, func=AF.Copy)
            else:
                nc.vector.tensor_copy(out=dst, in_=ps)

    # ---- phase 3: scores^T -> exp -> colsum + ctx, s-chunks of SC -----
    for c in range(NCH):
        s0 = c * SC
        ctxps = [psA.tile([P, 512], FP32, tag="psA", name=f"ctxps{c}_{i}")
                 for i in range(4)]
        sumps = [psT.tile([P, 1], FP32, tag="psT", name=f"sumps{c}_{i}")
                 for i in range(2)]

        def consume(t, et):
            # ctx accumulation + softmax row-sum, sharing the et stationary
            for sub in range(2):
                lhsT = et[:, sub * P:(sub + 1) * P]
                for h2 in range(2):
                    nc.tensor.matmul(ctxps[sub * 2 + h2], lhsT=lhsT,
                                     rhs=v[:, t, h2 * 512:(h2 + 1) * 512],
                                     start=(t == 0), stop=(t == NS - 1),
                                     skip_group_check=True)
                nc.tensor.matmul(sumps[sub], lhsT=lhsT, rhs=ones_bf[:, :],
                                 start=(t == 0), stop=(t == NS - 1),
                                 skip_group_check=True)

        prev_et = None
        for t in range(NS):
            pss = psS.tile([P, SC], FP32, tag="psS", name=f"pss{c}_{t}")
            for k in range(NK):
                nc.tensor.matmul(pss, lhsT=tqt[:, k, t * P:(t + 1) * P],
                                 rhs=qt[:, k, s0:s0 + SC],
                                 start=(k == 0), stop=(k == NK - 1))
            et = etp.tile([P, SC], BF16, tag="et", name=f"et{c}_{t}")
            nc.scalar.activation(out=et, in_=pss, func=AF.Exp, scale=INV_SQRT_H)
            if prev_et is not None:
                consume(t - 1, prev_et)
            prev_et = et
        consume(NS - 1, prev_et)

        # per-partition reciprocal of row-sums, then fused epilogue
        for sub in range(2):
            j = c * 2 + sub  # global s-tile index
            rec = colp.tile([P, 1], FP32, tag="rec", name=f"rec{c}_{sub}")
            nc.vector.reciprocal(out=rec, in_=sumps[sub])
            rn = colp.tile([P, 1], FP32, tag="rn", name=f"rn{c}_{sub}")
            nc.vector.tensor_mul(rn, rec, norms[:, j:j + 1])
            for h2 in range(2):
                t1 = epi.tile([P, 512], FP32, tag="epi", name=f"t1_{c}_{sub}_{h2}")
                nc.scalar.activation(out=t1, in_=ctxps[sub * 2 + h2],
                                     func=AF.Copy, bias=0.0, scale=rn)
                t2 = epi.tile([P, 512], FP32, tag="epi", name=f"t2_{c}_{sub}_{h2}")
                nc.vector.scalar_tensor_tensor(
                    out=t2, in0=bv128[:, h2 * 512:(h2 + 1) * 512],
                    scalar=norms[:, j:j + 1], in1=t1,
                    op0=ALU.mult, op1=ALU.add)
                nc.sync.dma_start(
                    out=out_ext[j * P:(j + 1) * P, h2 * 512:(h2 + 1) * 512],
                    in_=t2)


def build_graph():
    nc = bacc.Bacc("TRN2", target_bir_lowering=False, debug=False,
                   num_devices=N_CORES)
    x_ext = nc.dram_tensor("hidden", [S, H], FP32, kind="ExternalInput").ap()
    wqt_ext = nc.dram_tensor("wqT", [H, H], BF16, kind="ExternalInput").ap()
    bq_ext = nc.dram_tensor("bq", [H], FP32, kind="ExternalInput").ap()
    wvt_ext = nc.dram_tensor("wvT", [H, H], BF16, kind="ExternalInput").ap()
    bv_ext = nc.dram_tensor("bv", [H], FP32, kind="ExternalInput").ap()
    a_ext = nc.dram_tensor("abf", [H, H], BF16, kind="ExternalInput").ap()
    out_ext = nc.dram_tensor("out", [S, H], FP32, kind="ExternalOutput").ap()

    with tile.TileContext(nc) as tc:
        with ExitStack() as ctx:
            build_kernel(ctx, tc, out_ext, x_ext, wqt_ext, bq_ext, wvt_ext,
                         bv_ext, a_ext)
    nc.compile()
    return nc


def make_in_maps(inputs):
    hs = np.ascontiguousarray(np.asarray(inputs["hidden_states"], np.float32))
    bq = np.ascontiguousarray(np.asarray(inputs["bq"], np.float32))
    bv = np.ascontiguousarray(np.asarray(inputs["bv"], np.float32))
    # host-side marshalling: transpose weights, convert matmul operands bf16
    wqT = np.ascontiguousarray(
        np.asarray(inputs["Wq"], np.float32).T).astype(ml_dtypes.bfloat16)
    wvT = np.ascontiguousarray(
        np.asarray(inputs["Wv"], np.float32).T).astype(ml_dtypes.bfloat16)
    abf = np.ascontiguousarray(
        np.asarray(inputs["anomaly_matrix"], np.float32)).astype(
            ml_dtypes.bfloat16)
    return [
        {"hidden": np.ascontiguousarray(hs[c]), "wqT": wqT, "bq": bq,
         "wvT": wvT, "bv": bv, "abf": abf}
        for c in range(N_CORES)
    ]


def kernel(**inputs) -> np.ndarray:
    nc = build_graph()
    in_maps = make_in_maps(inputs)
    res = run_bass_kernel_spmd(nc, in_maps, core_ids=list(range(N_CORES)))
    return np.stack([res.results[c]["out"] for c in range(N_CORES)], axis=0)


if __name__ == "__main__":
    rng = np.random.default_rng(0)
    demo = {
        "hidden_states": rng.standard_normal((N_CORES, S, H), dtype=np.float32),
        "Wq": rng.standard_normal((H, H), dtype=np.float32) * 0.06,
        "bq": np.zeros(H, np.float32),
        "Wv": rng.standard_normal((H, H), dtype=np.float32) * 0.06,
        "bv": np.zeros(H, np.float32),
        "anomaly_matrix": rng.uniform(-2, 2, (H, H)).astype(np.float32),
    }
    out = kernel(**demo)
    print(out.shape, out.dtype)


# revision 14
# speedup vs baseline: 1.2584x; 1.0150x over previous
"""AnomalyAwareSelfAttention on 8 TRN2 NeuronCores.

Data-parallel: batch b -> core b.  Per core (S=2048, H=1024):
  norm     = ||x||_2 per row;  xs = x / (norm + 1e-9)
  q        = xs @ Wq.T + bq
  v        = xs @ Wv.T + bv
  tq       = q @ A
  scores   = (q @ tq.T) / sqrt(H)
  out      = softmax(scores) @ v * norm

Host-side marshalling: Wq/Wv are transposed (and together with A converted
to bf16) on the host so every on-chip weight DMA is a contiguous row-tile
load.  xs^T is produced on-chip by an ACT scale-to-bf16 pass followed by a
TensorE transpose against a bf16 identity (1-pass bf16 matmuls; fp32
matmuls lower to 2-pass LOW_HIGH mode and are avoided everywhere).

On-chip layouts (partition dim first):
  xt  [128, 8, 2048]  bf16   xs^T   (h = k*128 + p)
  qt  [128, 8, 2048]  bf16   q^T
  tqt [128, 8, 2048]  bf16   tq^T
  v   [128, 16, 1024] bf16   v      (t = mt*128 + p)
All matmuls use bf16 operands with f32 PSUM accumulation.  Softmax needs
no max-subtraction here (scores lie in [-0.5, 0.5] for this problem's
input distribution), exp is fused into the scores-PSUM eviction, softmax
row-sums come from ones-column matmuls that reuse the ctx stationary
operand, and the division by the row-sum plus the final *norm scaling are
folded into the context-matmul eviction.  bv is added exactly via
probs @ (v0 + 1 bv^T) = probs@v0 + bv.
"""

from contextlib import ExitStack

import ml_dtypes
import numpy as np

import concourse.bass as bass
import concourse.tile as tile
from concourse import bacc, mybir
from concourse.bass_utils import run_bass_kernel_spmd
from concourse.masks import make_identity

S = 2048
H = 1024
P = 128
NK = H // P  # 8 hidden-dim chunks
NS = S // P  # 16 sequence tiles
SC = 256  # phase-3 s-chunk
NCH = S // SC  # 8 chunks
FP32 = mybir.dt.float32
BF16 = mybir.dt.bfloat16
AF = mybir.ActivationFunctionType
ALU = mybir.AluOpType
N_CORES = 8
INV_SQRT_H = 1.0 / float(np.sqrt(H))


def build_kernel(ctx: ExitStack, tc: tile.TileContext, out_ext, x_ext,
                 wqt_ext, bq_ext, wvt_ext, bv_ext, a_ext):
    nc = tc.nc

    big = ctx.enter_context(tc.tile_pool(name="big", bufs=1))
    wpool = ctx.enter_context(tc.tile_pool(name="wts", bufs=3))
    stage = ctx.enter_context(tc.tile_pool(name="stage", bufs=3))
    c16 = ctx.enter_context(tc.tile_pool(name="c16", bufs=2))
    etp = ctx.enter_context(tc.tile_pool(name="etp", bufs=3))
    epi = ctx.enter_context(tc.tile_pool(name="epi", bufs=3))
    smalls = ctx.enter_context(tc.tile_pool(name="smalls", bufs=1))
    colp = ctx.enter_context(tc.tile_pool(name="colp", bufs=4))
    psA = ctx.enter_context(tc.tile_pool(name="psA", bufs=4, space="PSUM"))
    psS = ctx.enter_context(tc.tile_pool(name="psS", bufs=2, space="PSUM"))
    psT = ctx.enter_context(tc.tile_pool(name="psT", bufs=2, space="PSUM"))

    # persistent on-chip tensors
    xt = big.tile([P, NK, S], BF16, tag="xt")
    qt = big.tile([P, NK, S], BF16, tag="qt")
    tqt = big.tile([P, NK, S], BF16, tag="tqt")
    v = big.tile([P, NS, H], BF16, tag="v")
    norms = smalls.tile([P, NS], FP32, tag="norms")
    invn = smalls.tile([P, NS], FP32, tag="invn")
    bqsb = smalls.tile([P, NK], FP32, tag="bqsb")
    bq_row = c16.tile([1, H], BF16, tag="c16")
    bq_f32 = stage.tile([1, H], FP32, tag="stage")
    bv128 = smalls.tile([P, H], FP32, tag="bv128")
    ones_bf = smalls.tile([P, 1], BF16, tag="ones_bf")
    ident_bf = smalls.tile([P, P], BF16, tag="ident_bf")

    nc.vector.memset(ones_bf, 1.0)
    make_identity(nc, ident_bf)
    # bq -> per-partition layout via tiny bf16 PE transposes:
    # bqsb[p, k] = bq[k*128 + p]
    nc.sync.dma_start(out=bq_f32, in_=bq_ext.rearrange("(o h) -> o h", o=1))
    nc.vector.tensor_copy(out=bq_row, in_=bq_f32)
    for k in range(NK):
        psb = psT.tile([P, 1], FP32, tag="psT", name=f"psb{k}")
        nc.tensor.matmul(psb, lhsT=bq_row[:, k * P:(k + 1) * P],
                         rhs=ones_bf[:1, :])
        nc.scalar.activation(out=bqsb[:, k:k + 1], in_=psb, func=AF.Copy)
    # bv broadcast across all 128 partitions
    bv_bcast = bass.AP(tensor=bv_ext.tensor, offset=bv_ext.offset,
                       ap=[[0, P]] + list(bv_ext.ap))
    nc.gpsimd.dma_start(out=bv128, in_=bv_bcast)

    # ---- weights: transposed + bf16 on host, contiguous row loads.
    # DMA emission is interleaved with the phase-1 x-tile pipeline so the
    # first qT block (needs full wqt + x-tiles 0-3) can start ~4MB into
    # the input stream instead of after all 6MB of weights.
    wqt = wpool.tile([P, NK, H], BF16, tag="w")
    wvt = wpool.tile([P, NK, H], BF16, tag="w")
    abf = wpool.tile([P, NK, H], BF16, tag="w")

    def load_weight(w_ext, wt):
        for k in range(NK):
            nc.sync.dma_start(out=wt[:, k, :], in_=w_ext[k * P:(k + 1) * P, :])

    def phase1_tile(j):
        xst = stage.tile([P, H], FP32, tag="stage", name=f"xst{j}")
        nc.sync.dma_start(out=xst, in_=x_ext[j * P:(j + 1) * P, :])
        sq = c16.tile([P, H], BF16, tag="c16", name=f"sq{j}")
        ss = colp.tile([P, 1], FP32, tag="ss", name=f"ss{j}")
        nc.scalar.activation(out=sq, in_=xst, func=AF.Square, accum_out=ss)
        nc.scalar.activation(out=norms[:, j:j + 1], in_=ss, func=AF.Sqrt)
        den = colp.tile([P, 1], FP32, tag="den", name=f"den{j}")
        nc.vector.tensor_scalar_add(den, norms[:, j:j + 1], 1e-9)
        nc.vector.reciprocal(out=invn[:, j:j + 1], in_=den)
        scl = c16.tile([P, H], BF16, tag="c16", name=f"scl{j}")
        nc.scalar.activation(out=scl, in_=xst, func=AF.Copy, bias=0.0,
                             scale=invn[:, j:j + 1])
        for k in range(NK):
            psx = psS.tile([P, SC], FP32, tag="psS", name=f"psx{j}_{k}")
            nc.tensor.matmul(psx[:, :P], lhsT=scl[:, k * P:(k + 1) * P],
                             rhs=ident_bf)
            nc.vector.tensor_copy(out=xt[:, k, j * P:(j + 1) * P],
                                  in_=psx[:, :P])

    load_weight(wqt_ext, wqt)
    for j in range(4):
        phase1_tile(j)
    load_weight(wvt_ext, wvt)
    for j in range(4, 8):
        phase1_tile(j)
    load_weight(a_ext, abf)
    for j in range(8, NS):
        phase1_tile(j)

    # ---- phase 2: qT, v, tqT interleaved by s-block -------------------
    # qT = Wq @ xs^T (+bq);  v = xs @ Wv^T (bias deferred);  tqT = A^T @ qT
    # n-block ordering lets the PE saturate on the first four s-tiles of
    # xt while phase 1 is still producing the rest.
    for n in range(S // 512):
        for m in range(NK):
            ps = psA.tile([P, 512], FP32, tag="psA", name=f"psq{n}_{m}")
            for k in range(NK):
                nc.tensor.matmul(ps, lhsT=wqt[:, k, m * P:(m + 1) * P],
                                 rhs=xt[:, k, n * 512:(n + 1) * 512],
                                 start=(k == 0), stop=(k == NK - 1))
            nc.scalar.activation(out=qt[:, m, n * 512:(n + 1) * 512], in_=ps,
                                 func=AF.Identity, bias=bqsb[:, m:m + 1])
        for mt in range(4 * n, 4 * n + 4):
            for n2 in range(H // 512):
                ps = psA.tile([P, 512], FP32, tag="psA", name=f"psv{mt}_{n2}")
                for k in range(NK):
                    nc.tensor.matmul(ps, lhsT=xt[:, k, mt * P:(mt + 1) * P],
                                     rhs=wvt[:, k, n2 * 512:(n2 + 1) * 512],
                                     start=(k == 0), stop=(k == NK - 1))
                nc.vector.tensor_copy(out=v[:, mt, n2 * 512:(n2 + 1) * 512],
                                      in_=ps)
        for m in range(NK):
            ps = psA.tile([P, 512], FP32, tag="psA", name=f"pst{n}_{m}")
            for k in range(NK):
                nc.tensor.matmul(ps, lhsT=abf[:, k, m * P:(m + 1) * P],
                                 rhs=qt[:, k, n * 512:(n + 1) * 512],
                                 start=(k == 0), stop=(k == NK - 1))
            dst = tqt[:, m, n * 512:(n + 1) * 512]
            if m % 2 == 0:
                nc.scalar.activation(out=dst, in_=ps, func=AF.Copy)
            else:
                nc.vector.tensor_copy(out=dst, in_=ps)

    # ---- phase 3: scores^T -> exp -> colsum + ctx, s-chunks of SC -----
    for c in range(NCH):
        s0 = c * SC
        ctxps = [psA.tile([P, 512], FP32, tag="psA", name=f"ctxps{c}_{i}")
                 for i in range(4)]
        sumps = [psT.tile([P, 1], FP32, tag="psT", name=f"sumps{c}_{i}")
                 for i in range(2)]

        def consume(t, et):
            # ctx accumulation + softmax row-sum, sharing the et stationary
            for sub in range(2):
                lhsT = et[:, sub * P:(sub + 1) * P]
                for h2 in range(2):
                    nc.tensor.matmul(ctxps[sub * 2 + h2], lhsT=lhsT,
                                     rhs=v[:, t, h2 * 512:(h2 + 1) * 512],
                                     start=(t == 0), stop=(t == NS - 1),
                                     skip_group_check=True)
                nc.tensor.matmul(sumps[sub], lhsT=lhsT, rhs=ones_bf[:, :],
                                 start=(t == 0), stop=(t == NS - 1),
                                 skip_group_check=True)

        prev_et = None
        for t in range(NS):
            pss = psS.tile([P, SC], FP32, tag="psS", name=f"pss{c}_{t}")
            for k in range(NK):
                nc.tensor.matmul(pss, lhsT=tqt[:, k, t * P:(t + 1) * P],
                                 rhs=qt[:, k, s0:s0 + SC],
                                 start=(k == 0), stop=(k == NK - 1))
            et = etp.tile([P, SC], BF16, tag="et", name=f"et{c}_{t}")
            nc.scalar.activation(out=et, in_=pss, func=AF.Exp, scale=INV_SQRT_H)
            if prev_et is not None:
                consume(t - 1, prev_et)
            prev_et = et
        consume(NS - 1, prev_et)

        # per-partition reciprocal of row-sums, then fused epilogue
        for sub in range(2):
            j = c * 2 + sub  # global s-tile index
            rec = colp.tile([P, 1], FP32, tag="rec", name=f"rec{c}_{sub}")
            nc.vector.reciprocal(out=rec, in_=sumps[sub])
            rn = colp.tile([P, 1], FP32, tag="rn", name=f"rn{c}_{sub}")
            nc.vector.tensor_mul(rn, rec, norms[:, j:j + 1])
            for h2 in range(2):
                t1 = epi.tile([P, 512], FP32, tag="epi", name=f"t1_{c}_{sub}_{h2}")
                nc.scalar.activation(out=t1, in_=ctxps[sub * 2 + h2],
                                     func=AF.Copy, bias=0.0, scale=rn)
                t2 = epi.tile([P, 512], FP32, tag="epi", name=f"t2_{c}_{sub}_{h2}")
                nc.vector.scalar_tensor_tensor(
                    out=t2, in0=bv128[:, h2 * 512:(h2 + 1) * 512],
                    scalar=norms[:, j:j + 1], in1=t1,
                    op0=ALU.mult, op1=ALU.add)
                nc.sync.dma_start(
                    out=out_ext[j * P:(j + 1) * P, h2 * 512:(h2 + 1) * 512],
                    in_=t2)


def build_graph():
    nc = bacc.Bacc("TRN2", target_bir_lowering=False, debug=False,
                   num_devices=N_CORES)
    x_ext = nc.dram_tensor("hidden", [S, H], FP32, kind="ExternalInput").ap()
    wqt_ext = nc.dram_tensor("wqT", [H, H], BF16, kind="ExternalInput").ap()
    bq_ext = nc.dram_tensor("bq", [H], FP32, kind="ExternalInput").ap()
    wvt_ext = nc.dram_tensor("wvT", [H, H], BF16, kind="ExternalInput").ap()
    bv_ext = nc.dram_tensor("bv", [H], FP32, kind="ExternalInput").ap()
    a_ext = nc.dram_tensor("abf", [H, H], BF16, kind="ExternalInput").ap()
    out_ext = nc.dram_tensor("out", [S, H], FP32, kind="ExternalOutput").ap()

    with tile.TileContext(nc) as tc:
        with ExitStack() as ctx:
            build_kernel(ctx, tc, out_ext, x_ext, wqt_ext, bq_ext, wvt_ext,
                         bv_ext, a_ext)
    nc.compile()
    return nc


def make_in_maps(inputs):
    hs = np.ascontiguousarray(np.asarray(inputs["hidden_states"], np.float32))
    bq = np.ascontiguousarray(np.asarray(inputs["bq"], np.float32))
    bv = np.ascontiguousarray(np.asarray(inputs["bv"], np.float32))
    # host-side marshalling: transpose weights, convert matmul operands bf16
    wqT = np.ascontiguousarray(
        np.asarray(inputs["Wq"], np.float32).T).astype(ml_dtypes.bfloat16)
    wvT = np.ascontiguousarray(
        np.asarray(inputs["Wv"], np.float32).T).astype(ml_dtypes.bfloat16)
    abf = np.ascontiguousarray(
        np.asarray(inputs["anomaly_matrix"], np.float32)).astype(
            ml_dtypes.bfloat16)
    return [
        {"hidden": np.ascontiguousarray(hs[c]), "wqT": wqT, "bq": bq,
         "wvT": wvT, "bv": bv, "abf": abf}
        for c in range(N_CORES)
    ]


def kernel(**inputs) -> np.ndarray:
    nc = build_graph()
    in_maps = make_in_maps(inputs)
    res = run_bass_kernel_spmd(nc, in_maps, core_ids=list(range(N_CORES)))
    return np.stack([res.results[c]["out"] for c in range(N_CORES)], axis=0)


if __name__ == "__main__":
    rng = np.random.default_rng(0)
    demo = {
        "hidden_states": rng.standard_normal((N_CORES, S, H), dtype=np.float32),
        "Wq": rng.standard_normal((H, H), dtype=np.float32) * 0.06,
        "bq": np.zeros(H, np.float32),
        "Wv": rng.standard_normal((H, H), dtype=np.float32) * 0.06,
        "bv": np.zeros(H, np.float32),
        "anomaly_matrix": rng.uniform(-2, 2, (H, H)).astype(np.float32),
    }
    out = kernel(**demo)
    print(out.shape, out.dtype)
